# revision 5
# baseline (speedup 1.0000x reference)
"""Trainium2 Bass kernel for batched CRF forward algorithm (log-partition).

Reference computes, for feats [B,T,K] and transitions [K,K]:
    alpha_{t}[b,i] = logsumexp_j(alpha_{t-1}[b,j] + trans[i,j]) + feat_t[b,i]
    logZ[b] = logsumexp_i(alpha_{T-1}[b,i] + trans[STOP,i])

Device algorithm (exp domain):
    E_t = (Wf ^T @ E_{t-1}) * F_t   with Wf[j,i] = exp(trans[i,j]), F = exp(feat - C)
so the K-contraction is a TensorE matmul and the per-step elementwise
multiply is one DVE op.  A forward chain (t = 0..127) and a backward chain
(t = 255..128, state Bw_t[j] = exp(beta_t[j])) run concurrently and meet in
the middle:  Z[b] = sum_i Efwd_127[i,b] * Bw_127[i,b].
Every NORM steps each chain renormalizes per column (ones-matmul column sum
-> DVE reciprocal -> broadcast matmul -> multiply); the reciprocals are
written out and the host adds their logs back.

Layout per core: batch shard of 256 sequences packed as 2 groups x 48 tags
= 96 partitions x 128 columns; the two 48x48 weight blocks are block-diagonal
in a 96x96 lhsT.  Sharding: batch B=2048 split over 8 cores (data parallel,
transitions replicated), no collectives.
"""

import numpy as np

B, T, K = 2048, 256, 48
NCORE = 8
G = 2                    # tag-groups packed into the partition dim
PP = G * K               # 96 partitions
COLS = 128               # columns per tile (sequences per group per core)
S = T // 2               # 128 pair-steps (fwd + bwd)
NORM = 32                # renormalize every NORM steps per chain
CHUNK = 16               # pair-steps per DMA/exp chunk
NROUND = 2 * (S // NORM)  # 8 scale records total (4 fwd + 4 bwd)
BIAS_C = -4.4            # F = exp(feat + BIAS_C); host adds back -T*BIAS_C
START, STOP = 46, 47

_cache = {}


def _build():
    """Build the SPMD Bass program (identical on all 8 cores)."""
    import concourse.bass as bass
    import concourse.bacc as bacc
    import concourse.mybir as mybir
    from concourse import tile

    f32 = mybir.dt.float32
    bf16 = mybir.dt.bfloat16
    PSUM = bass.MemorySpace.PSUM
    Exp = mybir.ActivationFunctionType.Exp

    nc = bacc.Bacc(None, target_bir_lowering=False)

    feats = nc.dram_tensor("feats", [PP, S * 2 * COLS], f32, kind="ExternalInput")
    w2f = nc.dram_tensor("w2f", [PP, PP], bf16, kind="ExternalInput")
    w2b = nc.dram_tensor("w2b", [PP, PP], bf16, kind="ExternalInput")
    initf = nc.dram_tensor("initf", [PP, COLS], bf16, kind="ExternalInput")
    initb = nc.dram_tensor("initb", [PP, COLS], f32, kind="ExternalInput")
    ones2 = nc.dram_tensor("ones2", [PP, G], f32, kind="ExternalInput")
    bsel = nc.dram_tensor("bsel", [G, PP], f32, kind="ExternalInput")
    r_out = nc.dram_tensor("r_out", [NROUND, G, COLS], f32, kind="ExternalOutput")
    d_out = nc.dram_tensor("d_out", [G, COLS], f32, kind="ExternalOutput")

    with tile.TileContext(nc) as tc:
        with (
            tc.tile_pool(name="const", bufs=1) as cpool,
            tc.tile_pool(name="fraw", bufs=2) as fpool,
            tc.tile_pool(name="fexp", bufs=2) as epool,
            tc.tile_pool(name="state", bufs=3) as spool,
            tc.tile_pool(name="small", bufs=2) as rpool,
            tc.tile_pool(name="ps", bufs=2, space=PSUM) as pspool,
            tc.tile_pool(name="ps1", bufs=1, space=PSUM) as ps1pool,
        ):
            w2f_sb = cpool.tile([PP, PP], bf16, tag="w2f")
            w2b_sb = cpool.tile([PP, PP], bf16, tag="w2b")
            initf_sb = cpool.tile([PP, COLS], bf16, tag="initf")
            initb_sb = cpool.tile([PP, COLS], f32, tag="initb")
            ones2_sb = cpool.tile([PP, G], f32, tag="ones2")
            bsel_sb = cpool.tile([G, PP], f32, tag="bsel")
            nc.sync.dma_start(w2f_sb[:], w2f[:])
            nc.sync.dma_start(w2b_sb[:], w2b[:])
            nc.sync.dma_start(initf_sb[:], initf[:])
            nc.sync.dma_start(initb_sb[:], initb[:])
            nc.sync.dma_start(ones2_sb[:], ones2[:])
            nc.sync.dma_start(bsel_sb[:], bsel[:])
            bias_sb = cpool.tile([PP, 1], f32, tag="bias")
            nc.vector.memset(bias_sb[:], BIAS_C)

            ef = None          # fwd state, SBUF bf16 (None -> use initf_sb)
            bw_ps = None       # bwd state, PSUM f32 (None -> use initb_sb)
            ftile = None
            round_k = 0

            for s in range(S):
                if s % CHUNK == 0:
                    raw = fpool.tile([PP, CHUNK * 2 * COLS], f32, tag="raw")
                    c0 = s * 2 * COLS
                    nc.sync.dma_start(raw[:], feats[:, c0:c0 + CHUNK * 2 * COLS])
                    ftile = epool.tile([PP, CHUNK * 2 * COLS], f32, tag="fexp")
                    nc.scalar.activation(ftile[:], raw[:], Exp, bias=bias_sb[:])
                off = (s % CHUNK) * 2 * COLS
                f_fwd = ftile[:, off:off + COLS]
                f_bwd = ftile[:, off + COLS:off + 2 * COLS]

                # ---- forward chain: matmul then multiply ----
                pf = pspool.tile([PP, COLS], f32, tag="pf")
                rhs = initf_sb[:] if ef is None else ef[:]
                nc.tensor.matmul(pf[:], w2f_sb[:], rhs, start=True, stop=True)
                if s % NORM == NORM - 1:
                    tmp = spool.tile([PP, COLS], f32, tag="tmpf")
                    nc.vector.tensor_mul(tmp[:], pf[:], f_fwd)
                    sp = ps1pool.tile([G, COLS], f32, tag="sp")
                    nc.tensor.matmul(sp[:], ones2_sb[:], tmp[:], start=True, stop=True)
                    rr = rpool.tile([G, COLS], f32, tag="rr")
                    nc.vector.reciprocal(rr[:], sp[:])
                    nc.sync.dma_start(r_out[round_k], rr[:])
                    round_k += 1
                    rb = ps1pool.tile([PP, COLS], f32, tag="rb")
                    nc.tensor.matmul(rb[:], bsel_sb[:], rr[:], start=True, stop=True)
                    if s == S - 1:
                        ef = spool.tile([PP, COLS], f32, tag="effin")
                    else:
                        ef = spool.tile([PP, COLS], bf16, tag="ef")
                    nc.vector.tensor_mul(ef[:], tmp[:], rb[:])
                else:
                    ef = spool.tile([PP, COLS], bf16, tag="ef")
                    nc.vector.tensor_mul(ef[:], pf[:], f_fwd)

                # ---- backward chain: multiply then matmul ----
                bprev = initb_sb[:] if bw_ps is None else bw_ps[:]
                if s % NORM == NORM // 2 - 1:
                    gt = spool.tile([PP, COLS], f32, tag="gt")
                    nc.vector.tensor_mul(gt[:], bprev, f_bwd)
                    sp = ps1pool.tile([G, COLS], f32, tag="sp")
                    nc.tensor.matmul(sp[:], ones2_sb[:], gt[:], start=True, stop=True)
                    rr = rpool.tile([G, COLS], f32, tag="rr")
                    nc.vector.reciprocal(rr[:], sp[:])
                    nc.sync.dma_start(r_out[round_k], rr[:])
                    round_k += 1
                    rb = ps1pool.tile([PP, COLS], f32, tag="rb")
                    nc.tensor.matmul(rb[:], bsel_sb[:], rr[:], start=True, stop=True)
                    g = spool.tile([PP, COLS], bf16, tag="g")
                    nc.vector.tensor_mul(g[:], gt[:], rb[:])
                else:
                    g = spool.tile([PP, COLS], bf16, tag="g")
                    nc.vector.tensor_mul(g[:], bprev, f_bwd)
                bw_ps = pspool.tile([PP, COLS], f32, tag="bw")
                nc.tensor.matmul(bw_ps[:], w2b_sb[:], g[:], start=True, stop=True)

            # ---- middle combine: D[g,b] = sum_{i in group g} Ef[i,b]*Bw[i,b] ----
            comb = spool.tile([PP, COLS], f32, tag="comb")
            nc.vector.tensor_mul(comb[:], ef[:], bw_ps[:])
            dps = ps1pool.tile([G, COLS], f32, tag="dps")
            nc.tensor.matmul(dps[:], ones2_sb[:], comb[:], start=True, stop=True)
            dsb = rpool.tile([G, COLS], f32, tag="dsb")
            nc.vector.tensor_copy(dsb[:], dps[:])
            nc.sync.dma_start(d_out[:], dsb[:])

    nc.compile()
    return nc


def _pack_host(feats, transitions):
    """Host-side sharding/layout prep (numpy only)."""
    import ml_dtypes

    feats = np.asarray(feats, dtype=np.float32)
    trans = np.asarray(transitions, dtype=np.float32)

    # per-core packed feats: [core, p=(g,k), s*2*COLS + chain*COLS + col]
    x = feats.reshape(NCORE, G, COLS, T, K)
    fwd = x[:, :, :, :S, :]                       # t = s
    bwd = x[:, :, :, ::-1, :][:, :, :, :S, :]     # t = T-1-s
    pk = np.stack([fwd, bwd], axis=4)             # [core,g,col,s,chain,k]
    arr = pk.transpose(0, 1, 5, 3, 4, 2)          # [core,g,k,s,chain,col]
    feats_packed = np.ascontiguousarray(
        arr.reshape(NCORE, PP, S * 2 * COLS), dtype=np.float32)

    W = np.exp(trans.astype(np.float64))          # W[i,j] = exp(trans[i,j])
    Wf = W.T                                      # lhsT fwd: [j,i] = exp(trans[i,j])
    Wb = W                                        # lhsT bwd: [i,j] = exp(trans[i,j])
    w2f = np.zeros((PP, PP), dtype=np.float64)
    w2b = np.zeros((PP, PP), dtype=np.float64)
    for g in range(G):
        sl = slice(g * K, (g + 1) * K)
        w2f[sl, sl] = Wf
        w2b[sl, sl] = Wb
    w2f = w2f.astype(ml_dtypes.bfloat16)
    w2b = w2b.astype(ml_dtypes.bfloat16)

    initf = np.zeros((PP, COLS), dtype=np.float64)
    initf[START, :] = 1.0
    initf[K + START, :] = 1.0
    initf = initf.astype(ml_dtypes.bfloat16)

    wstop = np.exp(trans[STOP].astype(np.float64))   # [K]
    initb = np.tile(wstop[:, None], (G, COLS)).astype(np.float32)

    ones2 = np.zeros((PP, G), dtype=np.float32)
    for g in range(G):
        ones2[g * K:(g + 1) * K, g] = 1.0
    bsel = np.ascontiguousarray(ones2.T)

    shared = {"w2f": w2f, "w2b": w2b, "initf": initf, "initb": initb,
              "ones2": ones2, "bsel": bsel}
    return feats_packed, shared


def _postprocess(results):
    """Combine per-core device outputs into logZ [B] (float32)."""
    out = np.empty((NCORE, G, COLS), dtype=np.float64)
    for c in range(NCORE):
        d = np.asarray(results[c]["d_out"], dtype=np.float64)       # [G, COLS]
        rr = np.asarray(results[c]["r_out"], dtype=np.float64)      # [NROUND, G, COLS]
        out[c] = np.log(d) - np.log(rr).sum(axis=0) - T * BIAS_C
    return out.reshape(B).astype(np.float32)


def kernel(feats, transitions):
    from concourse.bass_utils import run_bass_kernel_spmd

    feats_packed, shared = _pack_host(feats, transitions)
    if "nc" not in _cache:
        _cache["nc"] = _build()
    nc = _cache["nc"]

    in_maps = [dict(shared, feats=feats_packed[c]) for c in range(NCORE)]
    res = run_bass_kernel_spmd(nc, in_maps, list(range(NCORE)))
    return _postprocess(res.results)
